# revision 1
# baseline (speedup 1.0000x reference)
"""ChunkedSparseAttention Trainium2 kernel.

Problem: B=2, S=4096, D=1024, CHUNK=64. Per chunk i:
  local  = softmax(Qi @ Ki^T / 32) @ Vi            (own 64 keys)
  cross  = softmax(Qi @ K[:64i]^T / 32) @ V[:64i]  (prefix keys)
  out_i  = local                   if i == 0
         = 0.9 * local + 0.1 * cross otherwise

Distribution: 8 cores, data-parallel over batch (4 cores/batch), with each
core taking one quad-chunk "group" (256 queries) from each of 4 classes
({0-3},{4-7},{8-11},{12-15}) so the triangular prefix work is balanced.
All cores run ONE SPMD NEFF: per-class kb loops are padded to the class max
and masked via a per-core bias table (exp(s/32 + bias), bias=-1e9 kills
padded key blocks). Per-core data differences are handled by host-side
gathers (queries, boundary keys/values, bias, blend coefficients).

On-chip layout ("S^T layout"): scores are computed keys-on-partitions,
S^T[k,q] = sum_d K^T[d,k] Q^T[d,q], so exp(S^T) is directly the lhsT of the
P@V matmul (no transposes on the critical path) and softmax denominators
come from a ones-column matmul. K^T/Q^T are pre-transposed on the host.
Matmuls run in float32r (full PE rate at N>=256, ~1.6e-4 rel err).
"""
import sys

for _p in ("/opt/trn_rl_repo", "/root/.axon_site/_ro/trn_rl_repo"):
    if _p not in sys.path:
        sys.path.insert(0, _p)

import numpy as np

import concourse.bass as bass
import concourse.mybir as mybir
import concourse.tile as tile
from concourse import bacc
from concourse.bass_utils import run_bass_kernel_spmd

F32 = mybir.dt.float32
F32R = mybir.dt.float32r
AF = mybir.ActivationFunctionType
SCALE = 1.0 / 32.0  # 1/sqrt(D)
NEG = -1e9


class Cfg:
    def __init__(self, S, classes):
        self.S = S
        self.D = 1024
        self.classes = classes            # list of 4 lists of group indices
        self.n_slot = len(classes)
        self.M = [2 * max(c) for c in classes]   # padded full-kb count per slot
        self.M = [max(m, 2) for m in self.M]
        self.maxM = max(self.M)
        self.GQ = 256                      # queries per group (4 chunks)
        self.NQ = self.n_slot * self.GQ    # queries per core
        self.n_dblk = self.D // 128
        self.cores_per_batch = len(classes[0])
        self.n_cores = 2 * self.cores_per_batch


FULL = Cfg(4096, [[0, 1, 2, 3], [4, 5, 6, 7], [8, 9, 10, 11], [12, 13, 14, 15]])
MINI = Cfg(1024, [[0], [1], [2], [3]])


def build_nc(cfg: Cfg):
    S, D = cfg.S, cfg.D
    NDB = cfg.n_dblk
    nc = bacc.Bacc("TRN2", target_bir_lowering=False, debug=False)

    kt_in = nc.dram_tensor("kt_in", [D, S], F32, kind="ExternalInput")
    qt_in = nc.dram_tensor("qt_in", [D, cfg.NQ], F32, kind="ExternalInput")
    kbt_in = nc.dram_tensor("kbt_in", [D, cfg.NQ], F32, kind="ExternalInput")
    v_in = nc.dram_tensor("v_in", [S, D], F32, kind="ExternalInput")
    vb_in = nc.dram_tensor("vb_in", [cfg.NQ, D], F32, kind="ExternalInput")
    bias_in = nc.dram_tensor("bias_in", [cfg.n_slot, 128, cfg.maxM], F32,
                             kind="ExternalInput")
    blend_in = nc.dram_tensor("blend_in", [cfg.n_slot, 128, 4], F32,
                              kind="ExternalInput")
    out_t = nc.dram_tensor("out_core", [cfg.NQ, D], F32, kind="ExternalOutput")
    dbg = getattr(cfg, "debug", False)
    if dbg:
        dbg_s = nc.dram_tensor("dbg_sums", [cfg.n_slot, 2, 128, 4], F32,
                               kind="ExternalOutput")
        dbg_o = nc.dram_tensor("dbg_o", [cfg.n_slot, 2, 128, D], F32,
                               kind="ExternalOutput")
    ones_dr = nc.inline_tensor(np.ones((128, 2), np.float32), "ones_c")

    with tile.TileContext(nc) as tc:
        with (
            tc.tile_pool(name="const", bufs=1) as cpool,
            tc.tile_pool(name="kt", bufs=1) as ktp,
            tc.tile_pool(name="qt", bufs=2) as qtp,
            tc.tile_pool(name="kbt", bufs=1) as kbtp,
            tc.tile_pool(name="vb", bufs=1) as vbp,
            tc.tile_pool(name="vsrc", bufs=3) as vsp,
            tc.tile_pool(name="vcast", bufs=3) as vcp,
            tc.tile_pool(name="et", bufs=4) as etp,
            tc.tile_pool(name="eb", bufs=3) as ebp,
            tc.tile_pool(name="bias", bufs=3) as biasp,
            tc.tile_pool(name="blend", bufs=2) as blp,
            tc.tile_pool(name="vec", bufs=10) as vecp,
            tc.tile_pool(name="outst", bufs=3) as outp,
            tc.tile_pool(name="poc", bufs=2, space="PSUM") as poc,
            tc.tile_pool(name="pst", bufs=2, space="PSUM") as pst,
            tc.tile_pool(name="psm", bufs=2, space="PSUM") as psm,
        ):
            ones_t = cpool.tile([128, 2], F32R)
            nc.gpsimd.dma_start(ones_t[:], ones_dr[:])
            ones_f32 = ones_t[:].bitcast(F32)

            # resident K^T, DMA-cast to f32r: [128(d), NDB, S]. Split along S
            # so early score matmuls only wait on the first column ranges
            # instead of the whole 16MB transfer.
            kt = ktp.tile([128, NDB, S], F32R)
            for i in range(8):
                c0, c1 = i * (S // 8), (i + 1) * (S // 8)
                nc.gpsimd.dma_start(
                    kt[:, :, c0:c1],
                    kt_in[:, c0:c1].rearrange("(db p) s -> p db s", p=128))

            for j in range(cfg.n_slot):
                Mj = cfg.M[j]
                qcol = j * cfg.GQ

                # per-slot Q^T, Kb^T (DMA-cast from host-transposed gathers)
                qt = qtp.tile([128, NDB, cfg.GQ], F32R)
                nc.gpsimd.dma_start(
                    qt[:], qt_in[:, qcol:qcol + cfg.GQ]
                    .rearrange("(db p) q -> p db q", p=128))
                kbt = kbtp.tile([128, NDB, cfg.GQ], F32R)
                nc.gpsimd.dma_start(
                    kbt[:], kbt_in[:, qcol:qcol + cfg.GQ]
                    .rearrange("(db p) q -> p db q", p=128))
                vb = vbp.tile([128, 2, D], F32R)
                nc.gpsimd.dma_start(
                    vb[:], vb_in[qcol:qcol + cfg.GQ, :]
                    .rearrange("(c p) d -> p c d", p=128))
                blend = blp.tile([128, 4], F32)
                nc.sync.dma_start(blend[:], blend_in[j])
                bias_slot = biasp.tile([128, cfg.maxM], F32)
                nc.sync.dma_start(bias_slot[:], bias_in[j])

                oc = [poc.tile([128, D], F32, tag="oc", name=f"oc{s}_{j}")
                      for s in range(2)]
                # one PSUM bank per accumulation chain: a second chain's
                # start=True in the same bank clobbers the first chain's
                # has_written state, so each sub's running sums gets its own
                # bank-padded tile.
                sums_c = [psm.tile([128, 2], F32, tag="sums", name=f"sc{s}_{j}")
                          for s in range(2)]

                # ---- full-kb loop (software-pipelined: QK(kb) then PV(kb-1))
                ets = {}
                vts = {}

                def emit_qk(kb):
                    vsrc = vsp.tile([128, D], F32)
                    nc.sync.dma_start(vsrc[:], v_in[kb * 128:(kb + 1) * 128, :])
                    vt = vcp.tile([128, D], F32R)
                    nc.vector.tensor_copy(vt[:], vsrc[:])
                    vts[kb] = vt
                    st = pst.tile([128, cfg.GQ], F32, tag="st")
                    for db in range(NDB):
                        nc.tensor.matmul(
                            st[:], kt[:, db, kb * 128:(kb + 1) * 128],
                            qt[:, db, :], start=(db == 0), stop=(db == NDB - 1))
                    et = etp.tile([128, cfg.GQ], F32R)
                    nc.scalar.activation(et[:], st[:], AF.Exp,
                                         bias=bias_slot[:, kb:kb + 1],
                                         scale=SCALE)
                    ets[kb] = et

                def emit_pv(kb):
                    et, vt = ets.pop(kb), vts.pop(kb)
                    for sub in range(2):
                        lhs = et[:, sub * 128:(sub + 1) * 128]
                        for dh in range(2):
                            nc.tensor.matmul(
                                oc[sub][:, dh * 512:(dh + 1) * 512], lhs,
                                vt[:, dh * 512:(dh + 1) * 512],
                                start=(kb == 0), stop=False)
                        nc.tensor.matmul(sums_c[sub][:], lhs,
                                         ones_t[:], start=(kb == 0), stop=False)

                for kb in range(Mj + 1):
                    if kb < Mj:
                        emit_qk(kb)
                    if kb >= 1:
                        emit_pv(kb - 1)

                # ---- boundary blocks b0/b1 (the group's own 256 keys)
                ebs = []
                for blk in range(2):
                    st = pst.tile([128, cfg.GQ], F32, tag="st")
                    for db in range(NDB):
                        nc.tensor.matmul(
                            st[:], kbt[:, db, blk * 128:(blk + 1) * 128],
                            qt[:, db, :], start=(db == 0), stop=(db == NDB - 1))
                    eb = ebp.tile([128, cfg.GQ], F32R)
                    nc.scalar.activation(eb[:], st[:], AF.Exp, scale=SCALE)
                    ebs.append(eb)
                eb0, eb1 = ebs

                # cross pieces within the boundary:
                # q1 (chunk 4g+1) <- first half of b0; dst partitions 64:128 -> fp32
                for dh in range(2):
                    nc.tensor.matmul(
                        oc[0][64:128, dh * 512:(dh + 1) * 512],
                        eb0[0:64, 64:128].bitcast(F32),
                        vb[0:64, 0, dh * 512:(dh + 1) * 512].bitcast(F32),
                        start=False, stop=(dh == 1))
                nc.tensor.matmul(sums_c[0][64:128, :],
                                 eb0[0:64, 64:128].bitcast(F32),
                                 ones_f32[0:64, :], start=False, stop=True)
                # q2,q3 <- all of b0; dst partitions 0:128 -> f32r
                for dh in range(2):
                    nc.tensor.matmul(
                        oc[1][:, dh * 512:(dh + 1) * 512],
                        eb0[:, 128:256], vb[:, 0, dh * 512:(dh + 1) * 512],
                        start=False, stop=False)
                nc.tensor.matmul(sums_c[1][:], eb0[:, 128:256], ones_t[:],
                                 start=False, stop=False)
                # q3 <- first half of b1; dst partitions 64:128 -> fp32
                for dh in range(2):
                    nc.tensor.matmul(
                        oc[1][64:128, dh * 512:(dh + 1) * 512],
                        eb1[0:64, 192:256].bitcast(F32),
                        vb[0:64, 1, dh * 512:(dh + 1) * 512].bitcast(F32),
                        start=False, stop=(dh == 1))
                nc.tensor.matmul(sums_c[1][64:128, :],
                                 eb1[0:64, 192:256].bitcast(F32),
                                 ones_f32[0:64, :], start=False, stop=True)

                # ---- flush cross, then local per sub (L reuses oc pool slots)
                sums_l = pst.tile([128, 4], F32, tag="st", name=f"sl_{j}")
                for sub in range(2):
                    eb = ebs[sub]
                    # cross normalization * alpha
                    scm = vecp.tile([128, 1], F32, tag="v")
                    nc.vector.tensor_scalar_max(
                        scm[:], sums_c[sub][:, 0:1], 1e-30)
                    rc = vecp.tile([128, 1], F32, tag="v")
                    nc.vector.reciprocal(rc[:], scm[:])
                    rc2 = vecp.tile([128, 1], F32, tag="v")
                    nc.vector.tensor_mul(rc2[:], rc[:],
                                         blend[:, 2 * sub + 1:2 * sub + 2])
                    cs = outp.tile([128, D], F32, tag="out")
                    nc.scalar.activation(cs[:], oc[sub][:], AF.Copy,
                                         scale=rc2[:])
                    if dbg:
                        dso = outp.tile([128, D], F32, tag="out")
                        nc.vector.tensor_copy(dso[:], oc[sub][:])
                        nc.sync.dma_start(dbg_o[j, sub], dso[:])
                        dss = vecp.tile([128, 2], F32, tag="dv", name="dss")
                        nc.vector.tensor_copy(dss[:], sums_c[sub][:])
                        nc.sync.dma_start(dbg_s[j, 0, :, 2 * sub:2 * sub + 2],
                                          dss[:])

                    # local attention for the two chunks of this sub
                    L = poc.tile([128, D], F32, tag="oc")
                    for dh in range(2):
                        nc.tensor.matmul(  # even chunk: partitions 0:64, f32r
                            L[0:64, dh * 512:(dh + 1) * 512],
                            eb[0:64, sub * 128:sub * 128 + 64],
                            vb[0:64, sub, dh * 512:(dh + 1) * 512],
                            start=True, stop=True)
                        nc.tensor.matmul(  # odd chunk: partitions 64:128, fp32
                            L[64:128, dh * 512:(dh + 1) * 512],
                            eb[64:128, sub * 128 + 64:sub * 128 + 128]
                            .bitcast(F32),
                            vb[64:128, sub, dh * 512:(dh + 1) * 512]
                            .bitcast(F32),
                            start=True, stop=True)
                    nc.tensor.matmul(sums_l[0:64, 2 * sub:2 * sub + 2],
                                     eb[0:64, sub * 128:sub * 128 + 64],
                                     ones_t[0:64, :], start=True, stop=True)
                    nc.tensor.matmul(sums_l[64:128, 2 * sub:2 * sub + 2],
                                     eb[64:128, sub * 128 + 64:sub * 128 + 128]
                                     .bitcast(F32),
                                     ones_f32[64:128, :], start=True, stop=True)

                    if dbg and sub == 1:
                        dsl = vecp.tile([128, 4], F32, tag="dv", name="dsl")
                        nc.vector.tensor_copy(dsl[:], sums_l[:])
                        nc.sync.dma_start(dbg_s[j, 1], dsl[:])
                    slm = vecp.tile([128, 1], F32, tag="v")
                    nc.vector.tensor_scalar_max(
                        slm[:], sums_l[:, 2 * sub:2 * sub + 1], 1e-30)
                    rl = vecp.tile([128, 1], F32, tag="v")
                    nc.vector.reciprocal(rl[:], slm[:])
                    rl2 = vecp.tile([128, 1], F32, tag="v")
                    nc.vector.tensor_mul(rl2[:], rl[:],
                                         blend[:, 2 * sub:2 * sub + 1])
                    lt = outp.tile([128, D], F32, tag="out")
                    nc.vector.tensor_scalar_mul(lt[:], L[:], rl2[:])
                    fin = outp.tile([128, D], F32, tag="out")
                    nc.vector.tensor_add(fin[:], lt[:], cs[:])
                    row = (2 * j + sub) * 128
                    nc.sync.dma_start(out_t[row:row + 128, :], fin[:])
    nc.compile()
    return nc


def _host_inputs(cfg: Cfg, query, key, value):
    """Build the 2*cores_per_batch per-core input maps."""
    in_maps = []
    for core in range(cfg.n_cores):
        b = core // cfg.cores_per_batch
        qt_idx = core % cfg.cores_per_batch
        groups = [cls[qt_idx] for cls in cfg.classes]
        kt_core = np.ascontiguousarray(key[b].T)
        q_rows = np.concatenate(
            [query[b, g * cfg.GQ:(g + 1) * cfg.GQ] for g in groups])
        kb_rows = np.concatenate(
            [key[b, g * cfg.GQ:(g + 1) * cfg.GQ] for g in groups])
        vb_rows = np.concatenate(
            [value[b, g * cfg.GQ:(g + 1) * cfg.GQ] for g in groups])
        bias = np.zeros((cfg.n_slot, 128, cfg.maxM), np.float32)
        blend = np.zeros((cfg.n_slot, 128, 4), np.float32)
        for j, g in enumerate(groups):
            bias[j, :, 2 * g:] = NEG
            for sub in range(2):
                for half in range(2):
                    chunk = 4 * g + 2 * sub + half
                    sl = slice(half * 64, half * 64 + 64)
                    blend[j, sl, 2 * sub] = 1.0 if chunk == 0 else 0.9
                    blend[j, sl, 2 * sub + 1] = 0.0 if chunk == 0 else 0.1
        in_maps.append({
            "kt_in": kt_core,
            "qt_in": np.ascontiguousarray(q_rows.T),
            "kbt_in": np.ascontiguousarray(kb_rows.T),
            "v_in": np.ascontiguousarray(value[b]),
            "vb_in": vb_rows,
            "bias_in": bias,
            "blend_in": blend,
        })
    return in_maps


def _scatter_output(cfg: Cfg, results, B):
    out = np.empty((B, cfg.S, cfg.D), np.float32)
    for core in range(cfg.n_cores):
        b = core // cfg.cores_per_batch
        qt_idx = core % cfg.cores_per_batch
        groups = [cls[qt_idx] for cls in cfg.classes]
        oc = results[core]["out_core"]
        for j, g in enumerate(groups):
            out[b, g * cfg.GQ:(g + 1) * cfg.GQ] = oc[j * cfg.GQ:(j + 1) * cfg.GQ]
    return out


_nc_cache = {}


def run(cfg: Cfg, query, key, value, trace=False, trace_kwargs=None):
    ck = (cfg.S, getattr(cfg, "debug", False))
    if ck not in _nc_cache:
        _nc_cache[ck] = build_nc(cfg)
    nc = _nc_cache[ck]
    in_maps = _host_inputs(cfg, query, key, value)
    kw = {}
    if trace:
        kw = dict(trace=True, trace_cores=list(range(cfg.n_cores)),
                  **(trace_kwargs or {}))
    res = run_bass_kernel_spmd(nc, in_maps, core_ids=list(range(cfg.n_cores)),
                               **kw)
    out = _scatter_output(cfg, res.results, query.shape[0])
    return out, res


def kernel(query, key, value):
    query = np.asarray(query, np.float32)
    key = np.asarray(key, np.float32)
    value = np.asarray(value, np.float32)
    out, _ = run(FULL, query, key, value)
    return out



# revision 10
# speedup vs baseline: 2.2002x; 2.2002x over previous
"""ChunkedSparseAttention Trainium2 kernel.

Problem: B=2, S=4096, D=1024, CHUNK=64. Per chunk i:
  local  = softmax(Qi @ Ki^T / 32) @ Vi            (own 64 keys)
  cross  = softmax(Qi @ K[:64i]^T / 32) @ V[:64i]  (prefix keys)
  out_i  = local                   if i == 0
         = 0.9 * local + 0.1 * cross otherwise

Distribution: 8 cores, data-parallel over batch (4 cores/batch), with each
core taking one quad-chunk "group" (256 queries) from each of 4 classes
({0-3},{4-7},{8-11},{12-15}) so the triangular prefix work is balanced.
All cores run ONE SPMD NEFF: per-class kb loops are padded to the class max
and masked via a per-core bias table (exp(s/32 + bias), bias=-1e9 kills
padded key blocks). Per-core data differences are handled by host-side
gathers (queries, boundary keys/values, bias, blend coefficients).

Precision/speed split:
  - The cross path (alpha=0.1) runs in fp8e4m3 with DoubleRow perf-mode
    matmuls: 256-deep contraction at 0.5 cycles/row = 4x the f32r MAC rate.
    Q/K/V are quantized on the host; P=exp(S) is quantized by the Act
    engine's fp8 output. The 0.1 blend weight keeps the fp8 noise well
    under the tolerance.
  - The local/boundary path (weight 0.9..1.0) runs in bf16.

On-chip layout ("S^T layout"): scores are computed keys-on-partitions,
S^T[k,q] = sum_d K^T[d,k] Q^T[d,q], so exp(S^T) is directly the lhsT of the
P@V matmul and softmax denominators come from a ones-column matmul.
K^T/Q^T are pre-transposed (and pre-quantized) on the host.
"""
import sys

for _p in ("/opt/trn_rl_repo", "/root/.axon_site/_ro/trn_rl_repo"):
    if _p not in sys.path:
        sys.path.insert(0, _p)

import numpy as np

import concourse.bass as bass
import concourse.mybir as mybir
import concourse.tile as tile
from concourse import bacc
from concourse.bass_utils import run_bass_kernel_spmd

F32 = mybir.dt.float32
F8 = mybir.dt.float8e4
BF16 = mybir.dt.bfloat16
DR = mybir.MatmulPerfMode.DoubleRow
AF = mybir.ActivationFunctionType
OP = mybir.AluOpType
SCALE = 1.0 / 32.0  # 1/sqrt(D)
NEG = -1e9
# Softmax-invariant logit shift: exp(s/32 + SHIFT) keeps the max weight
# (~e^6.9 unshifted) inside fp8e4m3's finite range; numerator and
# denominator scale together so the attention output is unchanged.
SHIFT = -2.5


class Cfg:
    def __init__(self, S, classes):
        self.S = S
        self.D = 1024
        self.classes = classes            # list of 4 lists of group indices
        self.n_slot = len(classes)
        self.M = [2 * max(c) for c in classes]   # padded full-kb count per slot
        self.M = [max(m, 2) for m in self.M]
        self.maxM = max(self.M)
        self.GQ = 256                      # queries per group (4 chunks)
        self.NQ = self.n_slot * self.GQ    # queries per core
        self.n_dblk = self.D // 128
        self.n_kblk = self.S // 128
        self.cores_per_batch = len(classes[0])
        self.n_cores = 2 * self.cores_per_batch


FULL = Cfg(4096, [[0, 1, 2, 3], [4, 5, 6, 7], [8, 9, 10, 11], [12, 13, 14, 15]])
MINI = Cfg(1024, [[0], [1], [2], [3]])


def build_nc(cfg: Cfg):
    S, D = cfg.S, cfg.D
    NDB = cfg.n_dblk
    NKB = cfg.n_kblk
    nc = bacc.Bacc("TRN2", target_bir_lowering=False, debug=False)

    kt8_in = nc.dram_tensor("kt8_in", [D, S], F8, kind="ExternalInput")
    qt8_in = nc.dram_tensor("qt8_in", [D, cfg.NQ], F8, kind="ExternalInput")
    v8_in = nc.dram_tensor("v8_in", [S, D], F8, kind="ExternalInput")
    qtb_in = nc.dram_tensor("qtb_in", [D, cfg.NQ], BF16, kind="ExternalInput")
    kbt_in = nc.dram_tensor("kbt_in", [D, cfg.NQ], BF16, kind="ExternalInput")
    vb_in = nc.dram_tensor("vb_in", [cfg.NQ, D], BF16, kind="ExternalInput")
    bias_in = nc.dram_tensor("bias_in", [cfg.n_slot, 128, cfg.maxM], F32,
                             kind="ExternalInput")
    blend_in = nc.dram_tensor("blend_in", [cfg.n_slot, 128, 4], F32,
                              kind="ExternalInput")
    out_t = nc.dram_tensor("out_core", [cfg.NQ, D], BF16, kind="ExternalOutput")
    ones8_dr = nc.inline_tensor(
        np.ones((128, 2, 2), mybir.dt.np(F8)), "ones8_c")
    onesb_dr = nc.inline_tensor(
        np.ones((128, 2), mybir.dt.np(BF16)), "onesb_c")

    with tile.TileContext(nc) as tc:
        with (
            tc.tile_pool(name="const", bufs=1) as cpool,
            tc.tile_pool(name="kt", bufs=1) as ktp,
            tc.tile_pool(name="v8", bufs=1) as v8p,
            tc.tile_pool(name="qk", bufs=1) as qkp,
            tc.tile_pool(name="et", bufs=4) as etp,
            tc.tile_pool(name="eb", bufs=3) as ebp,
            tc.tile_pool(name="bias", bufs=3) as biasp,
            tc.tile_pool(name="blend", bufs=2) as blp,
            tc.tile_pool(name="vec", bufs=10) as vecp,
            tc.tile_pool(name="outst", bufs=3) as outp,
            tc.tile_pool(name="poc", bufs=2, space="PSUM") as poc,
            tc.tile_pool(name="pst", bufs=2, space="PSUM") as pst,
            tc.tile_pool(name="psm", bufs=2, space="PSUM") as psm,
        ):
            ones8_t = cpool.tile([128, 2, 2], F8)
            nc.gpsimd.dma_start(ones8_t[:], ones8_dr[:])
            onesb_t = cpool.tile([128, 2], BF16)
            nc.gpsimd.dma_start(onesb_t[:], onesb_dr[:])
            shift_t = cpool.tile([128, 1], F32)
            nc.vector.memset(shift_t[:], SHIFT)

            # Resident K^T fp8 [128(d), NDB, S] and V fp8 [128(row), NKB, D].
            # Only the first maxM key blocks are ever used; loads are split
            # small-first and interleaved (K columns / V rows) so the first
            # QK+PV of the biggest slot start after ~1MB of traffic instead
            # of the full stream.
            maxcol = min(cfg.maxM * 128, S)
            kt8t = ktp.tile([128, NDB, S], F8)
            v8t = v8p.tile([128, NKB, D], F8)
            kcuts = [c for c in (0, 512, 1024, 2048, 3072, maxcol)
                     if c <= maxcol]
            vcuts = [b for b in (0, 2, 10, 18, cfg.maxM) if b <= cfg.maxM]
            loads = [("k", kcuts[i], kcuts[i + 1])
                     for i in range(len(kcuts) - 1)]
            vloads = [("v", vcuts[i], vcuts[i + 1])
                      for i in range(len(vcuts) - 1)]
            order = []
            for i in range(max(len(loads), len(vloads))):
                if i < len(loads):
                    order.append(loads[i])
                if i < len(vloads):
                    order.append(vloads[i])
            for kind, a, b in order:
                if kind == "k":
                    nc.gpsimd.dma_start(
                        kt8t[:, :, a:b],
                        kt8_in[:, a:b].rearrange("(db p) s -> p db s", p=128))
                else:
                    nc.gpsimd.dma_start(
                        v8t[:, a:b, :],
                        v8_in[a * 128:b * 128, :]
                        .rearrange("(kb p) d -> p kb d", p=128))

            qt8t = qkp.tile([128, NDB, cfg.NQ], F8)
            qtbt = qkp.tile([128, NDB, cfg.NQ], BF16)
            kbtt = qkp.tile([128, NDB, cfg.NQ], BF16)
            vbt = qkp.tile([128, 2 * cfg.n_slot, D], BF16)

            # biggest slot first: its long kb loop hides the rest of the
            # input stream, and the smallest slot leaves the shortest tail.
            slot_order = sorted(range(cfg.n_slot), key=lambda s: -cfg.M[s])
            for j in slot_order:
                Mj = cfg.M[j]
                Pj = Mj // 2
                qcol = j * cfg.GQ

                # per-slot input slices, urgent first (q for QK, bias for exp)
                nc.sync.dma_start(
                    qt8t[:, :, qcol:qcol + cfg.GQ],
                    qt8_in[:, qcol:qcol + cfg.GQ]
                    .rearrange("(db p) q -> p db q", p=128))
                bias_slot = biasp.tile([128, cfg.maxM], F32)
                nc.sync.dma_start(bias_slot[:], bias_in[j])
                nc.sync.dma_start(
                    kbtt[:, :, qcol:qcol + cfg.GQ],
                    kbt_in[:, qcol:qcol + cfg.GQ]
                    .rearrange("(db p) q -> p db q", p=128))
                nc.sync.dma_start(
                    qtbt[:, :, qcol:qcol + cfg.GQ],
                    qtb_in[:, qcol:qcol + cfg.GQ]
                    .rearrange("(db p) q -> p db q", p=128))
                nc.sync.dma_start(
                    vbt[:, 2 * j:2 * j + 2, :],
                    vb_in[qcol:qcol + cfg.GQ, :]
                    .rearrange("(c p) d -> p c d", p=128))
                blend = blp.tile([128, 4], F32)
                nc.sync.dma_start(blend[:], blend_in[j])

                oc = [poc.tile([128, D], F32, tag="oc", name=f"oc{s}_{j}")
                      for s in range(2)]
                # one PSUM bank per accumulation chain: a second chain's
                # start=True in the same bank clobbers the first chain's
                # has_written state, so each sub's running sums gets its own
                # bank-padded tile.
                sums_c = [psm.tile([128, 2], F32, tag="sums", name=f"sc{s}_{j}")
                          for s in range(2)]

                # ---- full-kb loop: fp8 DoubleRow (256-deep contraction).
                # QK(2t), QK(2t+1) then PV(pair t-1) so PE never waits on the
                # exp of the pair it is about to consume.
                ets = {}

                def emit_qk(kb):
                    pr, half = kb // 2, kb % 2
                    if half == 0:
                        ets[pr] = etp.tile([128, 2, cfg.GQ], F8, name=f"et_{j}")
                    st = pst.tile([128, cfg.GQ], F32, tag="st")
                    for t in range(NDB // 2):
                        nc.tensor.matmul(
                            st[:],
                            kt8t[:, 2 * t:2 * t + 2, kb * 128:(kb + 1) * 128],
                            qt8t[:, 2 * t:2 * t + 2, qcol:qcol + cfg.GQ],
                            start=(t == 0), stop=(t == NDB // 2 - 1),
                            perf_mode=DR)
                    nc.scalar.activation(ets[pr][:, half, :], st[:], AF.Exp,
                                         bias=bias_slot[:, kb:kb + 1],
                                         scale=SCALE)

                def emit_pv(pr):
                    et = ets.pop(pr)
                    for sub in range(2):
                        lhs = et[:, :, sub * 128:(sub + 1) * 128]
                        for dh in range(2):
                            nc.tensor.matmul(
                                oc[sub][:, dh * 512:(dh + 1) * 512], lhs,
                                v8t[:, 2 * pr:2 * pr + 2,
                                    dh * 512:(dh + 1) * 512],
                                start=(pr == 0), stop=False, perf_mode=DR)
                        nc.tensor.matmul(sums_c[sub][:], lhs, ones8_t[:],
                                         start=(pr == 0), stop=False,
                                         perf_mode=DR)

                for t in range(Pj + 1):
                    if t < Pj:
                        emit_qk(2 * t)
                        emit_qk(2 * t + 1)
                    if t >= 1:
                        emit_pv(t - 1)

                # ---- boundary blocks b0/b1 (the group's own 256 keys), bf16.
                # b1's scores are only needed for query cols 128:256, so its
                # QK runs at half width (eb1 col c == full col 128+c).
                widths = [cfg.GQ, cfg.GQ // 2]
                ebs = []
                for blk in range(2):
                    w = widths[blk]
                    qc0 = qcol + (cfg.GQ - w)
                    st = pst.tile([128, cfg.GQ], F32, tag="st")
                    kc = qcol + blk * 128
                    for db in range(NDB):
                        nc.tensor.matmul(
                            st[:, 0:w], kbtt[:, db, kc:kc + 128],
                            qtbt[:, db, qc0:qc0 + w],
                            start=(db == 0), stop=(db == NDB - 1))
                    eb = ebp.tile([128, cfg.GQ], BF16, name=f"eb{blk}_{j}")
                    nc.scalar.activation(eb[:, 0:w], st[:, 0:w], AF.Exp,
                                         bias=shift_t[:, 0:1], scale=SCALE)
                    ebs.append(eb)
                eb0, eb1 = ebs

                # local denominators early so their DVE readers clear the
                # 'st' PSUM slot before the next slot's QK needs it
                sums_l = pst.tile([128, 4], F32, tag="st", name=f"sl_{j}")
                rl2s = []
                for sub in range(2):
                    eb = ebs[sub]
                    off = (cfg.GQ - widths[sub])
                    nc.tensor.matmul(
                        sums_l[0:64, 2 * sub:2 * sub + 2],
                        eb[0:64, sub * 128 - off:sub * 128 - off + 64],
                        onesb_t[0:64, :], start=True, stop=True)
                    nc.tensor.matmul(
                        sums_l[64:128, 2 * sub:2 * sub + 2],
                        eb[64:128, sub * 128 + 64 - off:sub * 128 + 128 - off],
                        onesb_t[64:128, :], start=True, stop=True)
                    slm = vecp.tile([128, 1], F32, tag="v")
                    nc.vector.tensor_scalar_max(
                        slm[:], sums_l[:, 2 * sub:2 * sub + 1], 1e-30)
                    rl = vecp.tile([128, 1], F32, tag="v")
                    nc.vector.reciprocal(rl[:], slm[:])
                    rl2 = vecp.tile([128, 1], F32, tag="v", name=f"rl2_{j}{sub}")
                    nc.vector.tensor_mul(rl2[:], rl[:],
                                         blend[:, 2 * sub:2 * sub + 1])
                    rl2s.append(rl2)

                # cross pieces within the boundary:
                # q1 (chunk 4g+1) <- first half of b0; dst partitions 64:128
                for dh in range(2):
                    nc.tensor.matmul(
                        oc[0][64:128, dh * 512:(dh + 1) * 512],
                        eb0[0:64, 64:128],
                        vbt[0:64, 2 * j, dh * 512:(dh + 1) * 512],
                        start=False, stop=(dh == 1))
                nc.tensor.matmul(sums_c[0][64:128, :], eb0[0:64, 64:128],
                                 onesb_t[0:64, :], start=False, stop=True)
                # q2,q3 <- all of b0; dst partitions 0:128
                for dh in range(2):
                    nc.tensor.matmul(
                        oc[1][:, dh * 512:(dh + 1) * 512],
                        eb0[:, 128:256], vbt[:, 2 * j, dh * 512:(dh + 1) * 512],
                        start=False, stop=False)
                nc.tensor.matmul(sums_c[1][:], eb0[:, 128:256], onesb_t[:],
                                 start=False, stop=False)
                # q3 <- first half of b1; dst partitions 64:128
                # (narrow eb1 col c == full col 128 + c)
                for dh in range(2):
                    nc.tensor.matmul(
                        oc[1][64:128, dh * 512:(dh + 1) * 512],
                        eb1[0:64, 64:128],
                        vbt[0:64, 2 * j + 1, dh * 512:(dh + 1) * 512],
                        start=False, stop=(dh == 1))
                nc.tensor.matmul(sums_c[1][64:128, :], eb1[0:64, 64:128],
                                 onesb_t[0:64, :], start=False, stop=True)

                # ---- flush cross, then local per sub (L reuses oc pool slots)
                for sub in range(2):
                    eb = ebs[sub]
                    off = (cfg.GQ - widths[sub])
                    # cross normalization * alpha
                    scm = vecp.tile([128, 1], F32, tag="v")
                    nc.vector.tensor_scalar_max(
                        scm[:], sums_c[sub][:, 0:1], 1e-30)
                    rc = vecp.tile([128, 1], F32, tag="v")
                    nc.vector.reciprocal(rc[:], scm[:])
                    rc2 = vecp.tile([128, 1], F32, tag="v")
                    nc.vector.tensor_mul(rc2[:], rc[:],
                                         blend[:, 2 * sub + 1:2 * sub + 2])
                    cs = outp.tile([128, D], F32, tag="out")
                    nc.vector.tensor_scalar_mul(cs[:], oc[sub][:], rc2[:])

                    # local attention for the two chunks of this sub
                    L = poc.tile([128, D], F32, tag="oc")
                    for dh in range(2):
                        nc.tensor.matmul(  # even chunk: partitions 0:64
                            L[0:64, dh * 512:(dh + 1) * 512],
                            eb[0:64, sub * 128 - off:sub * 128 - off + 64],
                            vbt[0:64, 2 * j + sub, dh * 512:(dh + 1) * 512],
                            start=True, stop=True)
                        nc.tensor.matmul(  # odd chunk: partitions 64:128
                            L[64:128, dh * 512:(dh + 1) * 512],
                            eb[64:128,
                               sub * 128 + 64 - off:sub * 128 + 128 - off],
                            vbt[64:128, 2 * j + sub, dh * 512:(dh + 1) * 512],
                            start=True, stop=True)

                    fin = outp.tile([128, D], BF16, tag="outb")
                    nc.vector.scalar_tensor_tensor(
                        fin[:], L[:], rl2s[sub][:], cs[:], OP.mult, OP.add)
                    row = (2 * j + sub) * 128
                    nc.sync.dma_start(out_t[row:row + 128, :], fin[:])
    nc.compile()
    return nc


def _host_inputs(cfg: Cfg, query, key, value):
    """Build the 2*cores_per_batch per-core input maps."""
    f8 = mybir.dt.np(F8)
    bf = mybir.dt.np(BF16)
    in_maps = []
    per_batch = {}
    for b in range(query.shape[0]):
        per_batch[b] = {
            "kt8": np.ascontiguousarray(key[b].T).astype(f8),
            "v8": value[b].astype(f8),
        }
    for core in range(cfg.n_cores):
        b = core // cfg.cores_per_batch
        qt_idx = core % cfg.cores_per_batch
        groups = [cls[qt_idx] for cls in cfg.classes]
        q_rows = np.concatenate(
            [query[b, g * cfg.GQ:(g + 1) * cfg.GQ] for g in groups])
        kb_rows = np.concatenate(
            [key[b, g * cfg.GQ:(g + 1) * cfg.GQ] for g in groups])
        vb_rows = np.concatenate(
            [value[b, g * cfg.GQ:(g + 1) * cfg.GQ] for g in groups])
        qT = np.ascontiguousarray(q_rows.T)
        bias = np.full((cfg.n_slot, 128, cfg.maxM), SHIFT, np.float32)
        blend = np.zeros((cfg.n_slot, 128, 4), np.float32)
        for j, g in enumerate(groups):
            bias[j, :, 2 * g:] = NEG
            for sub in range(2):
                for half in range(2):
                    chunk = 4 * g + 2 * sub + half
                    sl = slice(half * 64, half * 64 + 64)
                    blend[j, sl, 2 * sub] = 1.0 if chunk == 0 else 0.9
                    blend[j, sl, 2 * sub + 1] = 0.0 if chunk == 0 else 0.1
        in_maps.append({
            "kt8_in": per_batch[b]["kt8"],
            "qt8_in": qT.astype(f8),
            "v8_in": per_batch[b]["v8"],
            "qtb_in": qT.astype(bf),
            "kbt_in": np.ascontiguousarray(kb_rows.T).astype(bf),
            "vb_in": vb_rows.astype(bf),
            "bias_in": bias,
            "blend_in": blend,
        })
    return in_maps


def _scatter_output(cfg: Cfg, results, B):
    out = np.empty((B, cfg.S, cfg.D), np.float32)
    for core in range(cfg.n_cores):
        b = core // cfg.cores_per_batch
        qt_idx = core % cfg.cores_per_batch
        groups = [cls[qt_idx] for cls in cfg.classes]
        oc = np.asarray(results[core]["out_core"], dtype=np.float32)
        for j, g in enumerate(groups):
            out[b, g * cfg.GQ:(g + 1) * cfg.GQ] = oc[j * cfg.GQ:(j + 1) * cfg.GQ]
    return out


_nc_cache = {}


def run(cfg: Cfg, query, key, value, trace=False, trace_kwargs=None):
    ck = (cfg.S, getattr(cfg, "debug", False))
    if ck not in _nc_cache:
        _nc_cache[ck] = build_nc(cfg)
    nc = _nc_cache[ck]
    in_maps = _host_inputs(cfg, query, key, value)
    kw = {}
    if trace:
        kw = dict(trace=True, trace_cores=list(range(cfg.n_cores)),
                  **(trace_kwargs or {}))
    res = run_bass_kernel_spmd(nc, in_maps, core_ids=list(range(cfg.n_cores)),
                               **kw)
    out = _scatter_output(cfg, res.results, query.shape[0])
    return out, res


def kernel(query, key, value):
    query = np.asarray(query, np.float32)
    key = np.asarray(key, np.float32)
    value = np.asarray(value, np.float32)
    out, _ = run(FULL, query, key, value)
    return out


# revision 23
# speedup vs baseline: 2.9471x; 1.3395x over previous
"""ChunkedSparseAttention Trainium2 kernel.

Problem: B=2, S=4096, D=1024, CHUNK=64. Per chunk i:
  local  = softmax(Qi @ Ki^T / 32) @ Vi            (own 64 keys)
  cross  = softmax(Qi @ K[:64i]^T / 32) @ V[:64i]  (prefix keys)
  out_i  = local                   if i == 0
         = 0.9 * local + 0.1 * cross otherwise

Distribution: 8 cores, data-parallel over batch (4 cores/batch), with each
core taking one quad-chunk "group" (256 queries) from each of 4 classes
({0-3},{4-7},{8-11},{12-15}) so the triangular prefix work is balanced.
All cores run ONE SPMD NEFF: per-class kb loops are padded to the class max
and masked via a per-core bias table (exp(s/32 + bias), bias=-1e9 kills
padded key blocks). Per-core data differences are handled by host-side
gathers (queries, boundary keys/values, bias, blend coefficients).

Precision/speed split:
  - The cross path (alpha=0.1) runs in fp8e4m3 with DoubleRow perf-mode
    matmuls: 256-deep contraction at 0.5 cycles/row = 4x the f32r MAC rate.
    Q/K/V are quantized on the host; P=exp(S) is quantized by the Act
    engine's fp8 output. The 0.1 blend weight keeps the fp8 noise well
    under the tolerance.
  - The local/boundary path (weight 0.9..1.0) runs in bf16.

On-chip layout ("S^T layout"): scores are computed keys-on-partitions,
S^T[k,q] = sum_d K^T[d,k] Q^T[d,q], so exp(S^T) is directly the lhsT of the
P@V matmul and softmax denominators come from a ones-column matmul.
K^T/Q^T are pre-transposed (and pre-quantized) on the host.
"""
import sys

for _p in ("/opt/trn_rl_repo", "/root/.axon_site/_ro/trn_rl_repo"):
    if _p not in sys.path:
        sys.path.insert(0, _p)

import numpy as np

import concourse.bass as bass
import concourse.mybir as mybir
import concourse.tile as tile
from concourse import bacc
from concourse.bass_utils import run_bass_kernel_spmd

F32 = mybir.dt.float32
F8 = mybir.dt.float8e4
F16 = mybir.dt.float16
DR = mybir.MatmulPerfMode.DoubleRow
AF = mybir.ActivationFunctionType
OP = mybir.AluOpType
SCALE = 1.0 / 32.0  # 1/sqrt(D)
NEG = -1e9
# Softmax-invariant logit shift: exp(s/32 + SHIFT) keeps the max weight
# (~e^6.9 unshifted) inside fp8e4m3's finite range; numerator and
# denominator scale together so the attention output is unchanged.
SHIFT = -2.5
BLOAD_BACK = 3


class Cfg:
    def __init__(self, S, classes):
        self.S = S
        self.D = 1024
        self.classes = classes            # list of 4 lists of group indices
        self.n_slot = len(classes)
        self.M = [2 * max(c) for c in classes]   # padded full-kb count per slot
        self.M = [max(m, 2) for m in self.M]
        self.maxM = max(self.M)
        self.GQ = 256                      # queries per group (4 chunks)
        self.NQ = self.n_slot * self.GQ    # queries per core
        self.n_dblk = self.D // 128
        self.n_kblk = self.S // 128
        self.cores_per_batch = len(classes[0])
        self.n_cores = 2 * self.cores_per_batch
        # processing order: smallest slot first, then descending size; the
        # host lays qt8 columns out in this order so one 512-col load covers
        # the first two slots
        by_size = sorted(range(self.n_slot), key=lambda s: -self.M[s])
        if self.n_slot >= 4:
            # smallest, 2nd-biggest, biggest, rest: keeps cumulative DMA
            # demand at or below the stream rate throughout
            self.slot_order = ([by_size[-1], by_size[1], by_size[0]]
                               + by_size[2:-1])
        else:
            self.slot_order = [by_size[-1]] + by_size[:-1]


FULL = Cfg(4096, [[0, 1, 2, 3], [4, 5, 6, 7], [8, 9, 10, 11], [12, 13, 14, 15]])
MINI = Cfg(1024, [[0], [1], [2], [3]])


def build_nc(cfg: Cfg):
    S, D = cfg.S, cfg.D
    NDB = cfg.n_dblk
    NKB = cfg.n_kblk
    nc = bacc.Bacc("TRN2", target_bir_lowering=False, debug=False)

    kt8_in = nc.dram_tensor("kt8_in", [D, S], F8, kind="ExternalInput")
    qt8_in = nc.dram_tensor("qt8_in", [D, cfg.NQ], F8, kind="ExternalInput")
    v8_in = nc.dram_tensor("v8_in", [S, D], F8, kind="ExternalInput")
    qtb_in = nc.dram_tensor("qtb_in", [D, cfg.NQ], F16, kind="ExternalInput")
    kbt_in = nc.dram_tensor("kbt_in", [D, cfg.NQ], F16, kind="ExternalInput")
    vb_in = nc.dram_tensor("vb_in", [cfg.NQ, D], F16, kind="ExternalInput")
    bias_in = nc.dram_tensor("bias_in", [cfg.n_slot, 128, cfg.maxM], F32,
                             kind="ExternalInput")
    blend_in = nc.dram_tensor("blend_in", [cfg.n_slot, 128, 4], F32,
                              kind="ExternalInput")
    out_t = nc.dram_tensor("out_core", [cfg.NQ, D], F16, kind="ExternalOutput")
    ones8_dr = nc.inline_tensor(
        np.ones((128, 2, 2), mybir.dt.np(F8)), "ones8_c")
    onesb_dr = nc.inline_tensor(
        np.ones((128, 2), mybir.dt.np(F16)), "onesb_c")

    with tile.TileContext(nc) as tc:
        with (
            tc.tile_pool(name="const", bufs=1) as cpool,
            tc.tile_pool(name="kt", bufs=1) as ktp,
            tc.tile_pool(name="v8", bufs=1) as v8p,
            tc.tile_pool(name="qk", bufs=1) as qkp,
            tc.tile_pool(name="et", bufs=4) as etp,
            tc.tile_pool(name="eb", bufs=3) as ebp,
            tc.tile_pool(name="vec", bufs=10) as vecp,
            tc.tile_pool(name="outst", bufs=3) as outp,
            tc.tile_pool(name="poc", bufs=2, space="PSUM") as poc,
            tc.tile_pool(name="pst", bufs=2, space="PSUM") as pst,
            tc.tile_pool(name="psm", bufs=2, space="PSUM") as psm,
        ):
            slot_order = cfg.slot_order
            qpos = {j: p for p, j in enumerate(slot_order)}

            # First SP DMA: the fp8 queries gating the first QK. qt8_in's
            # columns are host-permuted into slot_order, so one 512-col load
            # (full 512B rows, no small-element DMA penalty) covers the
            # first two processed slots; the rest loads mid-stream.
            qt8t = qkp.tile([128, NDB, cfg.NQ], F8)
            qhalf = min(2 * cfg.GQ, cfg.NQ)
            nc.sync.dma_start(
                qt8t[:, :, 0:qhalf],
                qt8_in[:, 0:qhalf].rearrange("(db p) q -> p db q", p=128))

            ones8_t = cpool.tile([128, 2, 2], F8)
            nc.sync.dma_start(ones8_t[:], ones8_dr[:])
            onesb_t = cpool.tile([128, 2], F16)
            nc.sync.dma_start(onesb_t[:], onesb_dr[:])
            shift_t = cpool.tile([128, 1], F32)
            nc.vector.memset(shift_t[:], SHIFT)
            bias_t = cpool.tile([128, cfg.n_slot, cfg.maxM], F32)
            nc.sync.dma_start(bias_t[:],
                              bias_in[:].rearrange("j p m -> p j m"))
            blend_t = cpool.tile([128, cfg.n_slot, 4], F32)
            nc.sync.dma_start(blend_t[:],
                              blend_in[:].rearrange("j p m -> p j m"))

            def load_qt8_rest():
                if qhalf < cfg.NQ:
                    nc.sync.dma_start(
                        qt8t[:, :, qhalf:cfg.NQ],
                        qt8_in[:, qhalf:cfg.NQ]
                        .rearrange("(db p) q -> p db q", p=128))

            # Resident K^T fp8 [128(d), NDB, S] and V fp8 [128(row), NKB, D].
            # Only the first maxM key blocks are ever used; loads are split
            # small-first and interleaved (K columns / V rows) so the first
            # QK+PV of the biggest slot start after ~1MB of traffic instead
            # of the full stream.
            maxcol = min(cfg.maxM * 128, S)
            kt8t = ktp.tile([128, NDB, S], F8)
            v8t = v8p.tile([128, NKB, D], F8)
            kcuts = sorted({c for c in (0, 256, 768, 1792, 2816, maxcol)
                            if c <= maxcol})
            vcuts = sorted({b for b in (0, 2, 6, 14, 22, cfg.maxM)
                            if b <= cfg.maxM})
            loads = [("k", kcuts[i], kcuts[i + 1])
                     for i in range(len(kcuts) - 1)]
            vloads = [("v", vcuts[i], vcuts[i + 1])
                      for i in range(len(vcuts) - 1)]
            order = []
            for i in range(max(len(loads), len(vloads))):
                if i < len(loads):
                    order.append(loads[i])
                if i < len(vloads):
                    order.append(vloads[i])
            for kind, a, b in order:
                if kind == "k":
                    nc.gpsimd.dma_start(
                        kt8t[:, :, a:b],
                        kt8_in[:, a:b].rearrange("(db p) s -> p db s", p=128))
                else:
                    nc.gpsimd.dma_start(
                        v8t[:, a:b, :],
                        v8_in[a * 128:b * 128, :]
                        .rearrange("(kb p) d -> p kb d", p=128))

            qtbt = qkp.tile([128, NDB, cfg.NQ], F16)
            kbtt = qkp.tile([128, NDB, cfg.NQ], F16)
            vbt = qkp.tile([128, 2 * cfg.n_slot, D], F16)

            for j in slot_order:
                Mj = cfg.M[j]
                Pj = Mj // 2
                qcol = j * cfg.GQ
                bias_slot = bias_t[:, j, :]
                blend = blend_t[:, j, :]

                def load_boundary(j=j, qcol=qcol):
                    # issued mid-kb-loop: early enough to beat the boundary
                    # matmuls, late enough not to starve the K/V stream
                    nc.sync.dma_start(
                        kbtt[:, :, qcol:qcol + cfg.GQ],
                        kbt_in[:, qcol:qcol + cfg.GQ]
                        .rearrange("(db p) q -> p db q", p=128))
                    nc.sync.dma_start(
                        qtbt[:, :, qcol:qcol + cfg.GQ],
                        qtb_in[:, qcol:qcol + cfg.GQ]
                        .rearrange("(db p) q -> p db q", p=128))
                    nc.sync.dma_start(
                        vbt[:, 2 * j:2 * j + 2, :],
                        vb_in[qcol:qcol + cfg.GQ, :]
                        .rearrange("(c p) d -> p c d", p=128))

                oc = [poc.tile([128, D], F32, tag="oc", name=f"oc{s}_{j}")
                      for s in range(2)]
                # one PSUM bank per accumulation chain: a second chain's
                # start=True in the same bank clobbers the first chain's
                # has_written state, so each sub's running sums gets its own
                # bank-padded tile.
                sums_c = [psm.tile([128, 2], F32, tag="sums", name=f"sc{s}_{j}")
                          for s in range(2)]

                # ---- full-kb loop: fp8 DoubleRow (256-deep contraction).
                # QK(2t), QK(2t+1) then PV(pair t-1) so PE never waits on the
                # exp of the pair it is about to consume.
                ets = {}

                def emit_qk(kb):
                    pr, half = kb // 2, kb % 2
                    if half == 0:
                        ets[pr] = etp.tile([128, 2, cfg.GQ], F8, name=f"et_{j}")
                    st = pst.tile([128, cfg.GQ], F32, tag="st")
                    for t in range(NDB // 2):
                        nc.tensor.matmul(
                            st[:],
                            kt8t[:, 2 * t:2 * t + 2, kb * 128:(kb + 1) * 128],
                            qt8t[:, 2 * t:2 * t + 2,
                                 qpos[j] * cfg.GQ:(qpos[j] + 1) * cfg.GQ],
                            start=(t == 0), stop=(t == NDB // 2 - 1),
                            perf_mode=DR)
                    nc.scalar.activation(ets[pr][:, half, :], st[:], AF.Exp,
                                         bias=bias_slot[:, kb:kb + 1],
                                         scale=SCALE)

                def emit_pv(pr):
                    et = ets.pop(pr)
                    for sub in range(2):
                        lhs = et[:, :, sub * 128:(sub + 1) * 128]
                        for dh in range(2):
                            nc.tensor.matmul(
                                oc[sub][:, dh * 512:(dh + 1) * 512], lhs,
                                v8t[:, 2 * pr:2 * pr + 2,
                                    dh * 512:(dh + 1) * 512],
                                start=(pr == 0), stop=False, perf_mode=DR)
                        nc.tensor.matmul(sums_c[sub][:], lhs, ones8_t[:],
                                         start=(pr == 0), stop=False,
                                         perf_mode=DR)

                bload_t = max(1, Pj - BLOAD_BACK)
                for t in range(Pj + 1):
                    if t < Pj:
                        emit_qk(2 * t)
                        emit_qk(2 * t + 1)
                    if t == 1 and j == slot_order[1]:
                        load_qt8_rest()
                    if t == bload_t:
                        load_boundary()
                    if t >= 1:
                        emit_pv(t - 1)

                # ---- boundary blocks b0/b1 (the group's own 256 keys), bf16.
                # b1's scores are only needed for query cols 128:256, so its
                # QK runs at half width (eb1 col c == full col 128+c).
                widths = [cfg.GQ, cfg.GQ // 2]
                ebs = []
                for blk in range(2):
                    w = widths[blk]
                    qc0 = qcol + (cfg.GQ - w)
                    st = pst.tile([128, cfg.GQ], F32, tag="st")
                    kc = qcol + blk * 128
                    for db in range(NDB):
                        nc.tensor.matmul(
                            st[:, 0:w], kbtt[:, db, kc:kc + 128],
                            qtbt[:, db, qc0:qc0 + w],
                            start=(db == 0), stop=(db == NDB - 1))
                    eb = ebp.tile([128, cfg.GQ], F16, name=f"eb{blk}_{j}")
                    nc.scalar.activation(eb[:, 0:w], st[:, 0:w], AF.Exp,
                                         bias=shift_t[:, 0:1], scale=SCALE)
                    ebs.append(eb)
                eb0, eb1 = ebs

                # local denominators early so their DVE readers clear the
                # 'st' PSUM slot before the next slot's QK needs it
                sums_l = pst.tile([128, 4], F32, tag="st", name=f"sl_{j}")
                rl2s = []
                for sub in range(2):
                    eb = ebs[sub]
                    off = (cfg.GQ - widths[sub])
                    nc.tensor.matmul(
                        sums_l[0:64, 2 * sub:2 * sub + 2],
                        eb[0:64, sub * 128 - off:sub * 128 - off + 64],
                        onesb_t[0:64, :], start=True, stop=True)
                    nc.tensor.matmul(
                        sums_l[64:128, 2 * sub:2 * sub + 2],
                        eb[64:128, sub * 128 + 64 - off:sub * 128 + 128 - off],
                        onesb_t[64:128, :], start=True, stop=True)
                    slm = vecp.tile([128, 1], F32, tag="v")
                    nc.vector.tensor_scalar_max(
                        slm[:], sums_l[:, 2 * sub:2 * sub + 1], 1e-30)
                    rl = vecp.tile([128, 1], F32, tag="v")
                    nc.vector.reciprocal(rl[:], slm[:])
                    rl2 = vecp.tile([128, 1], F32, tag="v", name=f"rl2_{j}{sub}")
                    nc.vector.tensor_mul(rl2[:], rl[:],
                                         blend[:, 2 * sub:2 * sub + 1])
                    rl2s.append(rl2)

                # cross pieces within the boundary:
                # q1 (chunk 4g+1) <- first half of b0; dst partitions 64:128
                for dh in range(2):
                    nc.tensor.matmul(
                        oc[0][64:128, dh * 512:(dh + 1) * 512],
                        eb0[0:64, 64:128],
                        vbt[0:64, 2 * j, dh * 512:(dh + 1) * 512],
                        start=False, stop=(dh == 1))
                nc.tensor.matmul(sums_c[0][64:128, :], eb0[0:64, 64:128],
                                 onesb_t[0:64, :], start=False, stop=True)
                # q2,q3 <- all of b0; dst partitions 0:128
                for dh in range(2):
                    nc.tensor.matmul(
                        oc[1][:, dh * 512:(dh + 1) * 512],
                        eb0[:, 128:256], vbt[:, 2 * j, dh * 512:(dh + 1) * 512],
                        start=False, stop=False)
                nc.tensor.matmul(sums_c[1][:], eb0[:, 128:256], onesb_t[:],
                                 start=False, stop=False)
                # q3 <- first half of b1; dst partitions 64:128
                # (narrow eb1 col c == full col 128 + c)
                for dh in range(2):
                    nc.tensor.matmul(
                        oc[1][64:128, dh * 512:(dh + 1) * 512],
                        eb1[0:64, 64:128],
                        vbt[0:64, 2 * j + 1, dh * 512:(dh + 1) * 512],
                        start=False, stop=(dh == 1))
                nc.tensor.matmul(sums_c[1][64:128, :], eb1[0:64, 64:128],
                                 onesb_t[0:64, :], start=False, stop=True)

                # ---- flush cross, then local per sub (L reuses oc pool slots)
                for sub in range(2):
                    eb = ebs[sub]
                    off = (cfg.GQ - widths[sub])
                    # cross normalization * alpha
                    scm = vecp.tile([128, 1], F32, tag="v")
                    nc.vector.tensor_scalar_max(
                        scm[:], sums_c[sub][:, 0:1], 1e-30)
                    rc = vecp.tile([128, 1], F32, tag="v")
                    nc.vector.reciprocal(rc[:], scm[:])
                    rc2 = vecp.tile([128, 1], F32, tag="v")
                    nc.vector.tensor_mul(rc2[:], rc[:],
                                         blend[:, 2 * sub + 1:2 * sub + 2])
                    cs = outp.tile([128, D], F32, tag="out")
                    nc.vector.tensor_scalar_mul(cs[:], oc[sub][:], rc2[:])

                    # local attention for the two chunks of this sub
                    L = poc.tile([128, D], F32, tag="oc")
                    for dh in range(2):
                        nc.tensor.matmul(  # even chunk: partitions 0:64
                            L[0:64, dh * 512:(dh + 1) * 512],
                            eb[0:64, sub * 128 - off:sub * 128 - off + 64],
                            vbt[0:64, 2 * j + sub, dh * 512:(dh + 1) * 512],
                            start=True, stop=True)
                        nc.tensor.matmul(  # odd chunk: partitions 64:128
                            L[64:128, dh * 512:(dh + 1) * 512],
                            eb[64:128,
                               sub * 128 + 64 - off:sub * 128 + 128 - off],
                            vbt[64:128, 2 * j + sub, dh * 512:(dh + 1) * 512],
                            start=True, stop=True)

                    fin = outp.tile([128, D], F16, tag="outb")
                    row = (2 * j + sub) * 128
                    for dh in range(2):
                        sl = slice(dh * 512, (dh + 1) * 512)
                        nc.vector.scalar_tensor_tensor(
                            fin[:, sl], L[:, sl], rl2s[sub][:], cs[:, sl],
                            OP.mult, OP.add)
                        nc.sync.dma_start(out_t[row:row + 128, sl],
                                          fin[:, sl])
    nc.compile()
    return nc


def _host_inputs(cfg: Cfg, query, key, value):
    """Build the 2*cores_per_batch per-core input maps."""
    f8 = mybir.dt.np(F8)
    f16 = np.float16
    in_maps = []
    per_batch = {}
    for b in range(query.shape[0]):
        per_batch[b] = {
            "kt8": np.ascontiguousarray(key[b].T).astype(f8),
            "v8": value[b].astype(f8),
        }
    for core in range(cfg.n_cores):
        b = core // cfg.cores_per_batch
        qt_idx = core % cfg.cores_per_batch
        groups = [cls[qt_idx] for cls in cfg.classes]
        q_rows = np.concatenate(
            [query[b, g * cfg.GQ:(g + 1) * cfg.GQ] for g in groups])
        kb_rows = np.concatenate(
            [key[b, g * cfg.GQ:(g + 1) * cfg.GQ] for g in groups])
        vb_rows = np.concatenate(
            [value[b, g * cfg.GQ:(g + 1) * cfg.GQ] for g in groups])
        # fp8 queries with columns permuted into processing order (the
        # kernel's first DMA covers the first two processed slots)
        q_perm = np.concatenate(
            [q_rows[j * cfg.GQ:(j + 1) * cfg.GQ] for j in cfg.slot_order])
        qT = np.ascontiguousarray(q_rows.T)
        bias = np.full((cfg.n_slot, 128, cfg.maxM), SHIFT, np.float32)
        blend = np.zeros((cfg.n_slot, 128, 4), np.float32)
        for j, g in enumerate(groups):
            bias[j, :, 2 * g:] = NEG
            for sub in range(2):
                for half in range(2):
                    chunk = 4 * g + 2 * sub + half
                    sl = slice(half * 64, half * 64 + 64)
                    blend[j, sl, 2 * sub] = 1.0 if chunk == 0 else 0.9
                    blend[j, sl, 2 * sub + 1] = 0.0 if chunk == 0 else 0.1
        in_maps.append({
            "kt8_in": per_batch[b]["kt8"],
            "qt8_in": np.ascontiguousarray(q_perm.T).astype(f8),
            "v8_in": per_batch[b]["v8"],
            "qtb_in": qT.astype(f16),
            "kbt_in": np.ascontiguousarray(kb_rows.T).astype(f16),
            "vb_in": vb_rows.astype(f16),
            "bias_in": bias,
            "blend_in": blend,
        })
    return in_maps


def _scatter_output(cfg: Cfg, results, B):
    out = np.empty((B, cfg.S, cfg.D), np.float32)
    for core in range(cfg.n_cores):
        b = core // cfg.cores_per_batch
        qt_idx = core % cfg.cores_per_batch
        groups = [cls[qt_idx] for cls in cfg.classes]
        oc = np.asarray(results[core]["out_core"], dtype=np.float32)
        for j, g in enumerate(groups):
            out[b, g * cfg.GQ:(g + 1) * cfg.GQ] = oc[j * cfg.GQ:(j + 1) * cfg.GQ]
    return out


_nc_cache = {}


def run(cfg: Cfg, query, key, value, trace=False, trace_kwargs=None):
    ck = (cfg.S, getattr(cfg, "debug", False))
    if ck not in _nc_cache:
        _nc_cache[ck] = build_nc(cfg)
    nc = _nc_cache[ck]
    in_maps = _host_inputs(cfg, query, key, value)
    kw = {}
    if trace:
        kw = dict(trace=True, trace_cores=list(range(cfg.n_cores)),
                  **(trace_kwargs or {}))
    res = run_bass_kernel_spmd(nc, in_maps, core_ids=list(range(cfg.n_cores)),
                               **kw)
    out = _scatter_output(cfg, res.results, query.shape[0])
    return out, res


def kernel(query, key, value):
    query = np.asarray(query, np.float32)
    key = np.asarray(key, np.float32)
    value = np.asarray(value, np.float32)
    out, _ = run(FULL, query, key, value)
    return out
